# revision 21
# baseline (speedup 1.0000x reference)
"""CxAM (context attention module) Trainium2 Bass kernel.

Full-input contract: kernel(**inputs) takes the unsharded tensors from
setup_inputs() and returns the full [16, 256, 64, 64] fp32 output.

Math (per sample, X = x[b] reshaped [C, H*W]):
    v      = Wv @ X + bv
    k_mean = mean_p(Wk @ X + bk)                           (mean commutes)
    att    = sigmoid((Wq^T k_mean)^T X + bq.k_mean)        (Q path collapses)
    out    = v * att[None, :]

and the whole attention-weight path collapses further on the host:
    w_eff = Wq^T k_mean = (Wq^T Wk/HW) xsum + Wq^T bk = M1 @ xsum + w0
    c     = bq.k_mean   = (bq^T Wk/HW) xsum + bq.bk   = r0 @ xsum + c0
so the device-side k-chain is: host-precomputed pixel-sum of x -> one tiny
matmul cluster -> one PSUM->SBUF hop -> logit. r0 is shipped pre-replicated
to [C,128] so c lands on all 128 PSUM partitions straight from the matmul.

Distribution: data-parallel over batch, 2 samples per NeuronCore x 8 cores.
fp16 x in / fp16 out halves HBM bytes (error budget is 2e-2 rel, this path
measures ~1e-3). The kernel is HBM-bound: per-core traffic is 4MB in + 4MB
out per iteration, so everything else must hide under the DMA stream.

v5 schedule (logit-first, fused drains, consolidated DMAs):
  - ONE 2MB DMA per sample for x ([128, 2*HW] tile, channel-chunk-major
    columns) and ONE 2MB DMA per sample for out; per-sample scalars
    (xsum replicated twice per channel chunk) ride in ONE tiny DMA.
    6 DMAs/iteration total vs 16 for per-tile transfers -- per-DMA fixed
    costs on the shared HBM path are a measurable fraction of the floor.
  - per sample the PE runs the tiny w_eff cluster (emitted one sample
    ahead), then logit chunks interleaved with the o=0 V chunks so the
    sigmoid (ScalarE) keeps pace without deep PSUM buffering, then the
    o=1 V chunks
  - each V PSUM chunk drains through ONE fused scalar_tensor_tensor
    (out_fp16 = (psum + bv) * att) on VectorE, except chunks 3 and 7
    which go unfused (ScalarE Identity+bias then GPSIMD SBUF multiply)
    so VectorE stays under the DMA floor
  - x loads on the sync queue, stores on the ScalarE queue (both HWDGE;
    the issuing engine is released after descriptor generation)
"""

import sys

sys.path.insert(0, "/opt/trn_rl_repo")

from contextlib import ExitStack

import numpy as np

import concourse.mybir as mybir
import concourse.tile as tile
from concourse import bacc
from concourse.bass_utils import run_bass_kernel_spmd

F32 = mybir.dt.float32
F32R = mybir.dt.float32r
FP16 = mybir.dt.float16
IODT = FP16
AF = mybir.ActivationFunctionType
ALU = mybir.AluOpType

B, C, H, W = 16, 256, 64, 64
HW = H * W
CR = 32
N_CORES = 8
BPC = B // N_CORES
NCH = 512            # logit matmul free-dim chunk (1 PSUM bank)
NP = HW // NCH
VCH = 1024           # V-path PSUM tile width (2 banks)
NPV = HW // VCH
CCH = C // 128
UNFUSED_CHUNKS = (1, 3, 5, 7)  # ACT(Identity+bias)->SBUF then GPSIMD TT*att
KTINY_AFTER = 3      # V chunks emitted before the next sample's tiny cluster

_CACHED_NC = None


def _build(rep=1):
    nc = bacc.Bacc("TRN2", target_bir_lowering=False, debug=False,
                   num_devices=N_CORES)

    # x / out as [BPC, 128, 2*HW]: row r of channel chunk cc of sample s
    # lives at column block cc of the per-sample [128, 2*HW] tile.
    xq_d = nc.dram_tensor("xq8", [BPC, 128, HW], mybir.dt.int8,
                          kind="ExternalInput").ap()
    x_d = nc.dram_tensor("x16", [BPC, 128, HW], IODT,
                         kind="ExternalInput").ap()
    out_d = nc.dram_tensor("out", [BPC, 128, CCH * HW], IODT,
                           kind="ExternalOutput").ap()
    wv_d = nc.dram_tensor("wvT16", [C, C], IODT, kind="ExternalInput").ap()
    m1_d = nc.dram_tensor("m1T", [C, C], F32R, kind="ExternalInput").ap()
    w0_d = nc.dram_tensor("w0r", [1, C + 128], F32R,
                          kind="ExternalInput").ap()
    r0_d = nc.dram_tensor("r0rep", [C, 128], F32R, kind="ExternalInput").ap()
    kv_d = nc.dram_tensor("kvec", [1, 4], F32R, kind="ExternalInput").ap()
    # per-sample scalars: [BPC, 128, 6]: cols 0:2 xsum2 of cc0, 2:4 of
    # cc1, col 4 the int8 scale of cc0 (col 5 pad)
    xs_d = nc.dram_tensor("xss6", [BPC, 128, 6], F32R,
                          kind="ExternalInput").ap()
    bv_d = nc.dram_tensor("bv2", [128, 2], F32, kind="ExternalInput").ap()

    with tile.TileContext(nc) as tc, ExitStack() as ctx:
        consts = ctx.enter_context(tc.tile_pool(name="consts", bufs=1))
        xin = ctx.enter_context(tc.tile_pool(name="xin", bufs=4))
        xqp = ctx.enter_context(tc.tile_pool(name="xqp", bufs=4))
        attp = ctx.enter_context(tc.tile_pool(name="att", bufs=2))
        outp = ctx.enter_context(tc.tile_pool(name="outp", bufs=2))
        small = ctx.enter_context(tc.tile_pool(name="small", bufs=8))
        pv = ctx.enter_context(tc.tile_pool(name="pv", bufs=2, space="PSUM"))
        pl = ctx.enter_context(tc.tile_pool(name="pl", bufs=3, space="PSUM"))
        pw = ctx.enter_context(tc.tile_pool(name="pw", bufs=1, space="PSUM"))
        vsb = ctx.enter_context(tc.tile_pool(name="vsb", bufs=4))

        wv = [consts.tile([128, C], IODT, tag=f"wv{i}", name=f"wv{i}")
              for i in range(CCH)]
        m1 = [consts.tile([128, C], F32R, tag=f"m1{i}", name=f"m1{i}")
              for i in range(CCH)]
        r0 = [consts.tile([128, 128], F32R, tag=f"r0{i}", name=f"r0{i}")
              for i in range(CCH)]
        for cc in range(CCH):
            nc.sync.dma_start(wv[cc][:], wv_d[cc * 128:(cc + 1) * 128, :])
            nc.sync.dma_start(m1[cc][:], m1_d[cc * 128:(cc + 1) * 128, :])
            nc.sync.dma_start(r0[cc][:], r0_d[cc * 128:(cc + 1) * 128, :])
        w0t = consts.tile([1, C + 128], F32R, tag="w0t")
        nc.sync.dma_start(w0t[:], w0_d[:])
        kvec = consts.tile([1, 4], F32R, tag="kvec")
        nc.sync.dma_start(kvec[:], kv_d[:])
        ones2 = kvec[0:1, 2:4]
        bv = consts.tile([128, 2], F32, tag="bv")
        nc.sync.dma_start(bv[:], bv_d[:])
        ones = consts.tile([128, 128], IODT, tag="ones")
        nc.vector.memset(ones[:], 1.0)

        def load_x(u, s):
            # channel chunk 0 rides as int8 (dequantized on VectorE two
            # samples ahead), chunk 1 as fp16 straight into the tile
            xt = xin.tile([128, CCH * HW], IODT, tag="x", name=f"xt{u}")
            xq = xqp.tile([128, HW], mybir.dt.int8, tag="xq",
                          name=f"xq{u}")
            nc.sync.dma_start(xq[:], xq_d[s])
            nc.sync.dma_start(xt[:, HW:2 * HW], x_d[s])
            xsb = small.tile([128, 6], F32R, tag="xsb", name=f"xsb{u}")
            nc.sync.dma_start(xsb[:], xs_d[s])
            return xt, xsb, xq

        def dequant_x(xt, xq, xsb):
            for pc in range(4):
                nc.vector.tensor_scalar(xt[:, pc * VCH:(pc + 1) * VCH],
                                        xq[:, pc * VCH:(pc + 1) * VCH],
                                        xsb[:, 4:5].bitcast(F32), None,
                                        ALU.mult)

        def tiny_cluster(u, xsb):
            # w_eff = M1 @ xsum + w0 (cols 0:4), c = r0.xsum + c0
            # replicated (cols 4:6)
            pwt = pw.tile([128, 6], F32, tag="pw", name=f"pw{u}")
            for ct in range(CCH):
                dst = pwt[:, 2 * ct:2 * ct + 2]
                for cc in range(CCH):
                    nc.tensor.matmul(dst, m1[cc][:, ct * 128:(ct + 1) * 128],
                                     xsb[:, 2 * cc:2 * cc + 2],
                                     start=(cc == 0), stop=False)
                nc.tensor.matmul(dst, w0t[0:1, ct * 128:(ct + 1) * 128],
                                 ones2, start=False, stop=True)
            for cc in range(CCH):
                nc.tensor.matmul(pwt[:, 4:6], r0[cc][:],
                                 xsb[:, 2 * cc:2 * cc + 2],
                                 start=(cc == 0), stop=False)
            nc.tensor.matmul(pwt[:, 4:6], w0t[0:1, C:C + 128], ones2,
                             start=False, stop=True)

            wsc = small.tile([128, 6], F32, tag="wsc", name=f"wsc{u}")
            nc.vector.tensor_copy(wsc[:], pwt[:])
            weff = [small.tile([128, 128], IODT, tag=f"weff{ct}",
                               name=f"weff{u}_{ct}")
                    for ct in range(CCH)]
            for ct in range(CCH):
                nc.vector.tensor_scalar(weff[ct][:], ones[:],
                                        wsc[:, 2 * ct:2 * ct + 1], None,
                                        ALU.mult)
            return wsc, weff

        samples = [(r, s) for r in range(rep) for s in range(BPC)]

        # prologue: load+dequant two samples ahead + sample 0's tiny
        xs_pipe = {}
        for j in range(min(2, len(samples))):
            uj = f"{samples[j][0]}_{samples[j][1]}"
            xs_pipe[j] = load_x(uj, samples[j][1])
            dequant_x(xs_pipe[j][0], xs_pipe[j][2], xs_pipe[j][1])
        cur_wk = tiny_cluster("0_0", xs_pipe[0][1])

        for idx, (r, s) in enumerate(samples):
            u = f"{r}_{s}"
            xt, xsb, _xq = xs_pipe.pop(idx)
            wsc, weff = cur_wk
            nxt = samples[idx + 1] if idx + 1 < len(samples) else None
            if nxt is not None:
                un = f"{nxt[0]}_{nxt[1]}"
            if idx + 2 < len(samples):
                u2 = f"{samples[idx + 2][0]}_{samples[idx + 2][1]}"
                xs_pipe[idx + 2] = load_x(u2, samples[idx + 2][1])

            att = attp.tile([128, HW], IODT, tag="att", name=f"att{u}")
            ot = outp.tile([128, CCH * HW], IODT, tag="ot", name=f"ot{u}")
            vchunk = 0

            def emit_logit(p):
                plt = pl.tile([128, NCH], F32, tag="pl", name=f"pl{u}_{p}")
                for ct in range(CCH):
                    nc.tensor.matmul(
                        plt[:], weff[ct][:],
                        xt[:, ct * HW + p * NCH:ct * HW + (p + 1) * NCH],
                        start=(ct == 0), stop=(ct == CCH - 1))
                nc.scalar.activation(att[:, p * NCH:(p + 1) * NCH],
                                     plt[:], AF.Sigmoid, bias=wsc[:, 4:5])

            def emit_v(o, p):
                nonlocal vchunk, cur_wk
                pvt = pv.tile([128, VCH], F32, tag="pv",
                              name=f"pv{u}_{o}_{p}")
                for half in range(2):
                    col = half * NCH
                    pcol = p * VCH + col
                    for cc in range(CCH):
                        nc.tensor.matmul(
                            pvt[:, col:col + NCH],
                            wv[cc][:, o * 128:(o + 1) * 128],
                            xt[:, cc * HW + pcol:cc * HW + pcol + NCH],
                            start=(cc == 0), stop=(cc == CCH - 1))
                dst = ot[:, o * HW + p * VCH:o * HW + (p + 1) * VCH]
                if vchunk in UNFUSED_CHUNKS:
                    vt = vsb.tile([128, VCH], IODT, tag="vt",
                                  name=f"vt{u}_{o}_{p}")
                    nc.scalar.activation(vt[:], pvt[:], AF.Identity,
                                         bias=bv[:, o:o + 1])
                    nc.gpsimd.tensor_mul(dst, vt[:],
                                         att[:, p * VCH:(p + 1) * VCH])
                else:
                    nc.vector.scalar_tensor_tensor(
                        dst, pvt[:], bv[:, o:o + 1],
                        att[:, p * VCH:(p + 1) * VCH], ALU.add, ALU.mult)
                vchunk += 1
                if vchunk == KTINY_AFTER and nxt is not None:
                    cur_wk = tiny_cluster(un, xs_pipe[idx + 1][1])

            for p in range(NPV):
                emit_logit(2 * p)
                emit_logit(2 * p + 1)
                emit_v(0, p)
                if p == NPV - 1:
                    nc.scalar.dma_start(out_d[s, :, 0:HW], ot[:, 0:HW])
            for p in range(NPV):
                emit_v(1, p)
            nc.gpsimd.dma_start(out_d[s, :, HW:2 * HW], ot[:, HW:2 * HW])
            if idx + 2 < len(samples):
                dequant_x(xs_pipe[idx + 2][0], xs_pipe[idx + 2][2],
                          xs_pipe[idx + 2][1])

    nc.compile()
    return nc


def _host_prep(Wq, bq, Wk, bk, Wv, bv):
    f16 = mybir.dt.np(IODT)
    Wq = np.asarray(Wq, np.float64)
    bq = np.asarray(bq, np.float64)
    Wk = np.asarray(Wk, np.float64) / HW
    bk = np.asarray(bk, np.float64)
    Wv = np.asarray(Wv, np.float32)
    bv = np.asarray(bv, np.float32)
    m1T = (Wk.T @ Wq).astype(np.float32)        # [C, C]: m1T[c,o]
    w0 = (Wq.T @ bk).astype(np.float32)         # [C]
    r0 = (Wk.T @ bq).astype(np.float32)         # [C]
    c0 = np.float32(bq @ bk)
    w0ext = np.concatenate([w0, np.full(128, c0, np.float32)])
    return {
        "wvT16": np.ascontiguousarray(Wv.T).astype(f16),
        "m1T": np.ascontiguousarray(m1T),
        "w0r": np.ascontiguousarray(w0ext[None, :]),
        "r0rep": np.ascontiguousarray(np.repeat(r0[:, None], 128, axis=1)),
        "kvec": np.array([[0.0, 0.0, 1.0, 1.0]], np.float32),
        "bv2": np.ascontiguousarray(bv.reshape(2, 128).T),
    }


def kernel(x, Wq, bq, Wk, bk, Wv, bv):
    global _CACHED_NC
    if _CACHED_NC is None:
        _CACHED_NC = _build()
    nc = _CACHED_NC

    f16 = mybir.dt.np(IODT)
    prep = _host_prep(Wq, bq, Wk, bk, Wv, bv)
    x = np.asarray(x, np.float32).reshape(B, C, HW)
    # channels 0:128 -> per-channel int8, channels 128:256 -> fp16
    xlo = x[:, :128, :]
    xsc = np.abs(xlo).max(axis=2, keepdims=True) / 127.0
    np.maximum(xsc, 1e-30, out=xsc)
    xq = np.clip(np.rint(xlo / xsc), -127, 127).astype(np.int8)
    xdq = xq.astype(np.float32) * xsc
    xsum0 = xdq.sum(axis=2, dtype=np.float64).astype(np.float32)
    xsum1 = x[:, 128:, :].sum(axis=2, dtype=np.float64).astype(np.float32)
    x16 = x[:, 128:, :].astype(f16)
    xss = np.stack([xsum0, xsum0, xsum1, xsum1,
                    xsc[:, :, 0].astype(np.float32), xsum1], axis=2)
    in_maps = []
    for core in range(N_CORES):
        sl = slice(core * BPC, (core + 1) * BPC)
        m = {"xq8": np.ascontiguousarray(xq[sl]),
             "x16": np.ascontiguousarray(x16[sl]),
             "xss6": np.ascontiguousarray(xss[sl])}
        m.update(prep)
        in_maps.append(m)

    res = run_bass_kernel_spmd(nc, in_maps, core_ids=list(range(N_CORES)))

    out = np.empty((B, C, HW), np.float32)
    for core in range(N_CORES):
        o = res.results[core]["out"].astype(np.float32)
        out[core * BPC:(core + 1) * BPC] = \
            o.reshape(BPC, 128, CCH, HW).transpose(0, 2, 1, 3).reshape(
                BPC, C, HW)
    return out.reshape(B, C, H, W)


# revision 22
# speedup vs baseline: 1.1973x; 1.1973x over previous
"""CxAM (context attention module) Trainium2 Bass kernel.

Full-input contract: kernel(**inputs) takes the unsharded tensors from
setup_inputs() and returns the full [16, 256, 64, 64] fp32 output.

Math (per sample, X = x[b] reshaped [C, H*W]):
    v      = Wv @ X + bv
    k_mean = mean_p(Wk @ X + bk)                           (mean commutes)
    att    = sigmoid((Wq^T k_mean)^T X + bq.k_mean)        (Q path collapses)
    out    = v * att[None, :]

and the whole attention-weight path collapses further on the host:
    w_eff = Wq^T k_mean = (Wq^T Wk/HW) xsum + Wq^T bk = M1 @ xsum + w0
    c     = bq.k_mean   = (bq^T Wk/HW) xsum + bq.bk   = r0 @ xsum + c0
so the device-side k-chain is: host-precomputed pixel-sum of x -> one tiny
matmul cluster -> one PSUM->SBUF hop -> logit. r0 is shipped pre-replicated
to [C,128] so c lands on all 128 PSUM partitions straight from the matmul.

Distribution: data-parallel over batch, 2 samples per NeuronCore x 8 cores.
fp16 x in / fp16 out halves HBM bytes (error budget is 2e-2 rel, this path
measures ~1e-3). The kernel is HBM-bound: per-core traffic is 4MB in + 4MB
out per iteration, so everything else must hide under the DMA stream.

v5 schedule (logit-first, fused drains, consolidated DMAs):
  - ONE 2MB DMA per sample for x ([128, 2*HW] tile, channel-chunk-major
    columns) and ONE 2MB DMA per sample for out; per-sample scalars
    (xsum replicated twice per channel chunk) ride in ONE tiny DMA.
    6 DMAs/iteration total vs 16 for per-tile transfers -- per-DMA fixed
    costs on the shared HBM path are a measurable fraction of the floor.
  - per sample the PE runs the tiny w_eff cluster (emitted one sample
    ahead), then logit chunks interleaved with the o=0 V chunks so the
    sigmoid (ScalarE) keeps pace without deep PSUM buffering, then the
    o=1 V chunks
  - each V PSUM chunk drains through ONE fused scalar_tensor_tensor
    (out_fp16 = (psum + bv) * att) on VectorE, except chunks 3 and 7
    which go unfused (ScalarE Identity+bias then GPSIMD SBUF multiply)
    so VectorE stays under the DMA floor
  - x loads on the sync queue, stores on the ScalarE queue (both HWDGE;
    the issuing engine is released after descriptor generation)
"""

import sys

sys.path.insert(0, "/opt/trn_rl_repo")

from contextlib import ExitStack

import numpy as np

import concourse.mybir as mybir
import concourse.tile as tile
from concourse import bacc
from concourse.bass_utils import run_bass_kernel_spmd

F32 = mybir.dt.float32
F32R = mybir.dt.float32r
FP16 = mybir.dt.float16
IODT = FP16
AF = mybir.ActivationFunctionType
ALU = mybir.AluOpType

B, C, H, W = 16, 256, 64, 64
HW = H * W
CR = 32
N_CORES = 8
BPC = B // N_CORES
NCH = 512            # logit matmul free-dim chunk (1 PSUM bank)
NP = HW // NCH
VCH = 1024           # V-path PSUM tile width (2 banks)
NPV = HW // VCH
CCH = C // 128
UNFUSED_CHUNKS = (3, 7)  # ACT(Identity+bias)->SBUF then GPSIMD TT*att
KTINY_AFTER = 3      # V chunks emitted before the next sample's tiny cluster

_CACHED_NC = None


def _build(rep=1):
    nc = bacc.Bacc("TRN2", target_bir_lowering=False, debug=False,
                   num_devices=N_CORES)

    # x / out as [BPC, 128, 2*HW]: row r of channel chunk cc of sample s
    # lives at column block cc of the per-sample [128, 2*HW] tile.
    x_d = nc.dram_tensor("x", [BPC, 128, CCH * HW], IODT,
                         kind="ExternalInput").ap()
    out_d = nc.dram_tensor("out", [BPC, 128, CCH * HW], IODT,
                           kind="ExternalOutput").ap()
    wv_d = nc.dram_tensor("wvT16", [C, C], IODT, kind="ExternalInput").ap()
    m1_d = nc.dram_tensor("m1T", [C, C], F32R, kind="ExternalInput").ap()
    w0_d = nc.dram_tensor("w0r", [1, C + 128], F32R,
                          kind="ExternalInput").ap()
    r0_d = nc.dram_tensor("r0rep", [C, 128], F32R, kind="ExternalInput").ap()
    kv_d = nc.dram_tensor("kvec", [1, 4], F32R, kind="ExternalInput").ap()
    # per-sample scalars: [BPC, 128, 4]: cols 0:2 xsum2 of cc0, 2:4 of cc1
    xs_d = nc.dram_tensor("xsum4", [BPC, 128, 2 * CCH], F32R,
                          kind="ExternalInput").ap()
    bv_d = nc.dram_tensor("bv2", [128, 2], F32, kind="ExternalInput").ap()

    with tile.TileContext(nc) as tc, ExitStack() as ctx:
        consts = ctx.enter_context(tc.tile_pool(name="consts", bufs=1))
        xin = ctx.enter_context(tc.tile_pool(name="xin", bufs=4))
        attp = ctx.enter_context(tc.tile_pool(name="att", bufs=2))
        outp = ctx.enter_context(tc.tile_pool(name="outp", bufs=2))
        small = ctx.enter_context(tc.tile_pool(name="small", bufs=8))
        pv = ctx.enter_context(tc.tile_pool(name="pv", bufs=2, space="PSUM"))
        pl = ctx.enter_context(tc.tile_pool(name="pl", bufs=3, space="PSUM"))
        pw = ctx.enter_context(tc.tile_pool(name="pw", bufs=1, space="PSUM"))
        vsb = ctx.enter_context(tc.tile_pool(name="vsb", bufs=4))

        wv = [consts.tile([128, C], IODT, tag=f"wv{i}", name=f"wv{i}")
              for i in range(CCH)]
        m1 = [consts.tile([128, C], F32R, tag=f"m1{i}", name=f"m1{i}")
              for i in range(CCH)]
        r0 = [consts.tile([128, 128], F32R, tag=f"r0{i}", name=f"r0{i}")
              for i in range(CCH)]
        for cc in range(CCH):
            nc.sync.dma_start(wv[cc][:], wv_d[cc * 128:(cc + 1) * 128, :])
            nc.sync.dma_start(m1[cc][:], m1_d[cc * 128:(cc + 1) * 128, :])
            nc.sync.dma_start(r0[cc][:], r0_d[cc * 128:(cc + 1) * 128, :])
        w0t = consts.tile([1, C + 128], F32R, tag="w0t")
        nc.sync.dma_start(w0t[:], w0_d[:])
        kvec = consts.tile([1, 4], F32R, tag="kvec")
        nc.sync.dma_start(kvec[:], kv_d[:])
        ones2 = kvec[0:1, 2:4]
        bv = consts.tile([128, 2], F32, tag="bv")
        nc.sync.dma_start(bv[:], bv_d[:])
        ones = consts.tile([128, 128], IODT, tag="ones")
        nc.vector.memset(ones[:], 1.0)

        def load_x(u, s):
            # per-sample tile loaded as two 1MB column-slice DMAs (8KB
            # descriptors, the measured sweet spot), plus one tiny DMA
            xt = xin.tile([128, CCH * HW], IODT, tag="x", name=f"xt{u}")
            for cc in range(CCH):
                nc.sync.dma_start(xt[:, cc * HW:(cc + 1) * HW],
                                  x_d[s, :, cc * HW:(cc + 1) * HW])
            xsb = small.tile([128, 2 * CCH], F32R, tag="xsb",
                             name=f"xsb{u}")
            nc.sync.dma_start(xsb[:], xs_d[s])
            return xt, xsb

        def tiny_cluster(u, xsb):
            # w_eff = M1 @ xsum + w0 (cols 0:4), c = r0.xsum + c0
            # replicated (cols 4:6)
            pwt = pw.tile([128, 6], F32, tag="pw", name=f"pw{u}")
            for ct in range(CCH):
                dst = pwt[:, 2 * ct:2 * ct + 2]
                for cc in range(CCH):
                    nc.tensor.matmul(dst, m1[cc][:, ct * 128:(ct + 1) * 128],
                                     xsb[:, 2 * cc:2 * cc + 2],
                                     start=(cc == 0), stop=False)
                nc.tensor.matmul(dst, w0t[0:1, ct * 128:(ct + 1) * 128],
                                 ones2, start=False, stop=True)
            for cc in range(CCH):
                nc.tensor.matmul(pwt[:, 4:6], r0[cc][:],
                                 xsb[:, 2 * cc:2 * cc + 2],
                                 start=(cc == 0), stop=False)
            nc.tensor.matmul(pwt[:, 4:6], w0t[0:1, C:C + 128], ones2,
                             start=False, stop=True)

            wsc = small.tile([128, 6], F32, tag="wsc", name=f"wsc{u}")
            nc.vector.tensor_copy(wsc[:], pwt[:])
            weff = [small.tile([128, 128], IODT, tag=f"weff{ct}",
                               name=f"weff{u}_{ct}")
                    for ct in range(CCH)]
            for ct in range(CCH):
                nc.vector.tensor_scalar(weff[ct][:], ones[:],
                                        wsc[:, 2 * ct:2 * ct + 1], None,
                                        ALU.mult)
            return wsc, weff

        samples = [(r, s) for r in range(rep) for s in range(BPC)]

        # prologue: load two samples ahead + sample 0's tiny cluster
        xs_pipe = {}
        for j in range(min(2, len(samples))):
            uj = f"{samples[j][0]}_{samples[j][1]}"
            xs_pipe[j] = load_x(uj, samples[j][1])
        cur_wk = tiny_cluster("0_0", xs_pipe[0][1])

        for idx, (r, s) in enumerate(samples):
            u = f"{r}_{s}"
            xt, xsb = xs_pipe.pop(idx)
            wsc, weff = cur_wk
            nxt = samples[idx + 1] if idx + 1 < len(samples) else None
            if nxt is not None:
                un = f"{nxt[0]}_{nxt[1]}"
            if idx + 2 < len(samples):
                u2 = f"{samples[idx + 2][0]}_{samples[idx + 2][1]}"
                xs_pipe[idx + 2] = load_x(u2, samples[idx + 2][1])

            att = attp.tile([128, HW], IODT, tag="att", name=f"att{u}")
            ot = outp.tile([128, CCH * HW], IODT, tag="ot", name=f"ot{u}")
            vchunk = 0

            def emit_logit(p):
                plt = pl.tile([128, NCH], F32, tag="pl", name=f"pl{u}_{p}")
                for ct in range(CCH):
                    nc.tensor.matmul(
                        plt[:], weff[ct][:],
                        xt[:, ct * HW + p * NCH:ct * HW + (p + 1) * NCH],
                        start=(ct == 0), stop=(ct == CCH - 1))
                nc.scalar.activation(att[:, p * NCH:(p + 1) * NCH],
                                     plt[:], AF.Sigmoid, bias=wsc[:, 4:5])

            def emit_v(o, p):
                nonlocal vchunk, cur_wk
                pvt = pv.tile([128, VCH], F32, tag="pv",
                              name=f"pv{u}_{o}_{p}")
                for half in range(2):
                    col = half * NCH
                    pcol = p * VCH + col
                    for cc in range(CCH):
                        nc.tensor.matmul(
                            pvt[:, col:col + NCH],
                            wv[cc][:, o * 128:(o + 1) * 128],
                            xt[:, cc * HW + pcol:cc * HW + pcol + NCH],
                            start=(cc == 0), stop=(cc == CCH - 1))
                dst = ot[:, o * HW + p * VCH:o * HW + (p + 1) * VCH]
                if vchunk in UNFUSED_CHUNKS:
                    vt = vsb.tile([128, VCH], IODT, tag="vt",
                                  name=f"vt{u}_{o}_{p}")
                    nc.scalar.activation(vt[:], pvt[:], AF.Identity,
                                         bias=bv[:, o:o + 1])
                    nc.gpsimd.tensor_mul(dst, vt[:],
                                         att[:, p * VCH:(p + 1) * VCH])
                else:
                    nc.vector.scalar_tensor_tensor(
                        dst, pvt[:], bv[:, o:o + 1],
                        att[:, p * VCH:(p + 1) * VCH], ALU.add, ALU.mult)
                vchunk += 1
                if vchunk == KTINY_AFTER and nxt is not None:
                    cur_wk = tiny_cluster(un, xs_pipe[idx + 1][1])

            for p in range(NPV):
                emit_logit(2 * p)
                emit_logit(2 * p + 1)
                emit_v(0, p)
                if p == NPV - 1:
                    nc.scalar.dma_start(out_d[s, :, 0:HW], ot[:, 0:HW])
            for p in range(NPV):
                emit_v(1, p)
            nc.gpsimd.dma_start(out_d[s, :, HW:2 * HW], ot[:, HW:2 * HW])

    nc.compile()
    return nc


def _host_prep(Wq, bq, Wk, bk, Wv, bv):
    f16 = mybir.dt.np(IODT)
    Wq = np.asarray(Wq, np.float64)
    bq = np.asarray(bq, np.float64)
    Wk = np.asarray(Wk, np.float64) / HW
    bk = np.asarray(bk, np.float64)
    Wv = np.asarray(Wv, np.float32)
    bv = np.asarray(bv, np.float32)
    m1T = (Wk.T @ Wq).astype(np.float32)        # [C, C]: m1T[c,o]
    w0 = (Wq.T @ bk).astype(np.float32)         # [C]
    r0 = (Wk.T @ bq).astype(np.float32)         # [C]
    c0 = np.float32(bq @ bk)
    w0ext = np.concatenate([w0, np.full(128, c0, np.float32)])
    return {
        "wvT16": np.ascontiguousarray(Wv.T).astype(f16),
        "m1T": np.ascontiguousarray(m1T),
        "w0r": np.ascontiguousarray(w0ext[None, :]),
        "r0rep": np.ascontiguousarray(np.repeat(r0[:, None], 128, axis=1)),
        "kvec": np.array([[0.0, 0.0, 1.0, 1.0]], np.float32),
        "bv2": np.ascontiguousarray(bv.reshape(2, 128).T),
    }


def kernel(x, Wq, bq, Wk, bk, Wv, bv):
    global _CACHED_NC
    if _CACHED_NC is None:
        _CACHED_NC = _build()
    nc = _CACHED_NC

    f16 = mybir.dt.np(IODT)
    prep = _host_prep(Wq, bq, Wk, bk, Wv, bv)
    x = np.asarray(x, np.float32).reshape(B, C, HW)
    xsum = x.sum(axis=2, dtype=np.float64).astype(np.float32)   # [B, C]
    x = x.astype(f16)
    # [B, 128, 4]: per channel-row, xsum2 of cc0 then cc1
    xs4 = np.stack([xsum[:, :128], xsum[:, :128],
                    xsum[:, 128:], xsum[:, 128:]], axis=2)
    in_maps = []
    for core in range(N_CORES):
        sl = slice(core * BPC, (core + 1) * BPC)
        xc = x[sl].reshape(BPC, CCH, 128, HW).transpose(0, 2, 1, 3)
        m = {"x": np.ascontiguousarray(xc.reshape(BPC, 128, CCH * HW)),
             "xsum4": np.ascontiguousarray(xs4[sl])}
        m.update(prep)
        in_maps.append(m)

    res = run_bass_kernel_spmd(nc, in_maps, core_ids=list(range(N_CORES)))

    out = np.empty((B, C, HW), np.float32)
    for core in range(N_CORES):
        o = res.results[core]["out"].astype(np.float32)
        out[core * BPC:(core + 1) * BPC] = \
            o.reshape(BPC, 128, CCH, HW).transpose(0, 2, 1, 3).reshape(
                BPC, C, HW)
    return out.reshape(B, C, H, W)
